# revision 13
# baseline (speedup 1.0000x reference)
"""Two-layer LSTM (Keras-faithful) Bass kernel for TRN2, 8 NeuronCores.

Problem: x [2048, 128, 128] -> LSTM(64, tanh) -> LSTM(64, sigmoid)
         -> output [2048, 8192, 1] (h2 sequence reshaped).

Strategy
--------
- Pure data parallelism: batch 2048 -> 8 cores x 256. Weights replicated.
- Per core, the 256-batch is split into 2 independent streams of 128 so the
  serial per-timestep dependency chain of one stream overlaps with the other
  stream's work on different engines.
- Both layers are fused into one scan over t. PSUM gate tile layout per
  stream-step: S [128, 4*NB] fp32 with column blocks [i | f | g | o] (NB=128
  batch columns each); rows 0:64 = layer-1 gate units, rows 64:128 = layer-2
  gate units. This makes every elementwise tensor_tensor operand pair live on
  identical partitions, so each DVE op processes both layers at once.
- Matmuls (fp16 inputs, fp32 PSUM accumulate), per gate g:
    S[0:64,  g] = W1g^T @ x_t            (K=128, M=64)  start
    S[0:64,  g]+= U1g^T @ h1             (K=64,  M=64)  accum
    S[64:128,g] = [W2g;U2g]^T @ [h1;h2]  (K=128, M=64)  start+stop
- All-sigmoid trick: layer-1's cell activation is tanh but tanh(z)=2*sigmoid(2z)-1.
  The g-columns of W1/U1 (+bias) are pre-scaled by 2 on the host, so ONE
  Sigmoid over the whole [128, 4*NB] PSUM tile computes every gate of both
  layers; a per-partition affine (a=[2|1], b=[-1|0]) via dual-op tensor_scalar
  recovers tanh on the layer-1 half. The same constants fix up sigmoid(2*c1)
  -> tanh(c1) after the cell activation (done with per-partition scale=[2|1]).
- h state stored fp16 [h1; h2] stacked (direct matmul rhs for both layers),
  c state fp32. Layer-2 h output streams to DRAM each step; the host undoes
  the layout transposes.

The kernel is self-contained: shapes/sharding hardcoded.
"""

import numpy as np

B, T, D, H = 2048, 128, 128, 64
NCORES = 8
BL = B // NCORES  # 256 batch rows per core
NS = 2            # independent streams per core
NB = BL // NS     # 128 batch rows per stream
G4 = 4 * H        # 256 gate units per layer

_CACHE = {}

# test.py can flip this before calling kernel() to capture an NTFF profile.
TRACE = False
LAST_EXEC_NS = None


def _split_multi_waits(bir_json: bytes) -> bytes:
    """This image's walrus codegen rejects instructions carrying more than one
    sync-wait command.  Split any multi-wait instruction into single-wait NoOps
    on the same engine (program order makes the waits happen-before the op)."""
    import json

    bj = json.loads(bir_json)
    changed = False
    for fn in bj.get("functions", []):
        for bb in fn.get("blocks", []):
            insts = bb.get("instructions")
            if not insts:
                continue
            out = []
            for ins in insts:
                si = ins.get("sync_info")
                waits = (si or {}).get("on_wait") or []
                if len(waits) > 1:
                    changed = True
                    for k, extra in enumerate(waits[:-1]):
                        out.append(
                            {
                                "debug": ins.get("debug", 0),
                                "engine": ins["engine"],
                                "ins": [],
                                "outs": [],
                                "name": f"{ins['name']}-sw{k}",
                                "opcode": "NoOp",
                                "sync_info": {"on_wait": [extra], "on_update": []},
                            }
                        )
                    si["on_wait"] = waits[-1:]
                out.append(ins)
            bb["instructions"] = out
    if not changed:
        return bir_json
    return json.dumps(bj).encode()


_PATCHED = False


def _install_wait_split_patch():
    global _PATCHED
    if _PATCHED:
        return
    from concourse import bass2jax, bass_utils

    orig = bass_utils.compile_bir_kernel

    def patched(bir_json, tmpdir, neff_name="file.neff"):
        return orig(_split_multi_waits(bir_json), tmpdir, neff_name)

    bass_utils.compile_bir_kernel = patched
    bass2jax.compile_bir_kernel = patched
    _PATCHED = True


def _install_ntff_shim():
    """The agent image's ``antenv`` lacks ``axon_hooks``, so boot-time NTFF
    hook registration silently degrades.  Recreate the tiny registry module
    and register the ctypes-based hook from ``trn_agent_boot`` so
    ``run_bass_kernel_spmd(trace=True)`` can capture profiles."""
    import sys
    import types

    if "antenv.axon_hooks" in sys.modules:
        return
    try:
        import antenv
        from trn_agent_boot.trn_boot import _ntff_profile_via_ctypes

        hook = _ntff_profile_via_ctypes("/opt/axon/libaxon_pjrt.so")
        mod = types.ModuleType("antenv.axon_hooks")
        _reg = {"hook": hook}
        mod.set_axon_ntff_profile_hook = lambda h: _reg.__setitem__("hook", h)
        mod.get_axon_ntff_profile_hook = lambda: _reg["hook"]
        sys.modules["antenv.axon_hooks"] = mod
        antenv.axon_hooks = mod
    except Exception as e:  # tracing is best-effort; grading never uses it
        print(f"ntff shim unavailable: {e}")


def _build(has_bias: bool):
    from contextlib import ExitStack

    import concourse.bass as bass
    import concourse.tile as tile
    from concourse import mybir

    fp32 = mybir.dt.float32
    fp16 = mybir.dt.float16
    AF = mybir.ActivationFunctionType
    OP = mybir.AluOpType

    nc = bass.Bass(trn_type="TRN2")

    xt_d = nc.dram_tensor("xt", [NS, T, D, NB], fp16, kind="ExternalInput")
    # Packed weights: cols 0:256 = W1 (g-cols x2), 256:512 = U1 (rows 0:64),
    # 512:768 = [W2; U2].  One DMA -> one semaphore for all weight consumers.
    wts_d = nc.dram_tensor("wts", [128, 3 * G4], fp16, kind="ExternalInput")
    bias_d = None
    if has_bias:
        bias_d = nc.dram_tensor("bias", [128, 4], fp32, kind="ExternalInput")
    out_d = nc.dram_tensor("out", [NS, T, H, NB], fp16, kind="ExternalOutput")

    with ExitStack() as ctx:
        tc = ctx.enter_context(tile.TileContext(nc))
        wpool = ctx.enter_context(tc.tile_pool(name="w", bufs=1))
        spool = ctx.enter_context(tc.tile_pool(name="S", bufs=4, space="PSUM"))
        xpool = ctx.enter_context(tc.tile_pool(name="x", bufs=6))
        gpool = ctx.enter_context(tc.tile_pool(name="g", bufs=2))
        epool = ctx.enter_context(tc.tile_pool(name="e", bufs=3))
        hpool = ctx.enter_context(tc.tile_pool(name="h", bufs=3))

        wts = wpool.tile([128, 3 * G4], fp16, tag="wts")
        nc.sync.dma_start(wts[:], wts_d[:, :])
        w1 = wts[:, 0:G4]
        u1 = wts[0:64, G4 : 2 * G4]
        v2 = wts[:, 2 * G4 : 3 * G4]

        # Affine fixup constants, built with DVE memsets (same engine as the
        # tensor_scalar consumers -> no cross-engine semaphores).
        cst = wpool.tile([128, 2], fp32, tag="cst")
        nc.vector.memset(cst[0:64, 0:1], 2.0)
        nc.vector.memset(cst[64:128, 0:1], 1.0)
        nc.vector.memset(cst[0:64, 1:2], -1.0)
        nc.vector.memset(cst[64:128, 1:2], 0.0)
        a_ap = cst[:, 0:1]
        b_ap = cst[:, 1:2]
        bias_t = None
        if has_bias:
            bias_t = wpool.tile([128, 4], fp32, tag="bias")
            nc.sync.dma_start(bias_t[:], bias_d[:, :])

        h_prev = []
        c_prev = []
        for s in range(NS):
            h0 = hpool.tile([128, NB], fp16, tag=f"h{s}")
            nc.vector.memset(h0[:], 0.0)
            c0 = epool.tile([128, NB], fp32, tag=f"c{s}")
            nc.vector.memset(c0[:], 0.0)
            h_prev.append(h0)
            c_prev.append(c0)

        # Layer 2 lags layer 1 by one step: iteration `it` computes L1 step
        # `it` (rows 0:64) and L2 step `it-1` (rows 64:128).  The h/c tiles
        # carried between iterations are exactly [h1[it-1]; h2[it-2]], which
        # is what both layers' matmuls need.  Iteration T runs the final L2
        # step only.
        for it in range(T + 1):
            for s in range(NS):
                S = spool.tile([128, 4 * NB], fp32, tag=f"S{s}")
                hp = h_prev[s]
                if it < T:
                    xt = xpool.tile([D, NB], fp16, tag=f"x{s}")
                    nc.sync.dma_start(xt[:], xt_d[s, it, :, :])
                for g in range(4):
                    blk = S[:, g * NB : (g + 1) * NB]
                    gc = slice(g * H, (g + 1) * H)
                    if it < T:
                        nc.tensor.matmul(
                            blk[0:64, :], w1[:, gc], xt[:], start=True, stop=False
                        )
                        nc.tensor.matmul(
                            blk[0:64, :], u1[:, gc], hp[0:64, :], start=False, stop=True
                        )
                    else:
                        nc.tensor.matmul(
                            blk[0:64, :], u1[:, gc], hp[0:64, :], start=True, stop=True
                        )
                    nc.tensor.matmul(
                        blk[64:128, :], v2[:, gc], hp[:, :], start=True, stop=True
                    )

                gs = gpool.tile([128, 4 * NB], fp32, tag=f"gs{s}")
                if has_bias:
                    for g in range(4):
                        nc.scalar.activation(
                            gs[:, g * NB : (g + 1) * NB],
                            S[:, g * NB : (g + 1) * NB],
                            AF.Sigmoid,
                            bias=bias_t[:, g : g + 1],
                        )
                else:
                    nc.scalar.activation(gs[:], S[:], AF.Sigmoid)

                i_ap = gs[:, 0:NB]
                f_ap = gs[:, NB : 2 * NB]
                g_ap = gs[:, 2 * NB : 3 * NB]
                o_ap = gs[:, 3 * NB : 4 * NB]

                geff = epool.tile([128, NB], fp32, tag=f"geff{s}")
                nc.vector.tensor_scalar(geff[:], g_ap, a_ap, b_ap, OP.mult, OP.add)
                ut = epool.tile([128, NB], fp32, tag=f"ut{s}")
                nc.vector.tensor_tensor(ut[:], i_ap, geff[:], OP.mult)
                m = epool.tile([128, NB], fp32, tag=f"m{s}")
                nc.vector.tensor_tensor(m[:], f_ap, c_prev[s][:], OP.mult)
                cn = epool.tile([128, NB], fp32, tag=f"c{s}")
                nc.gpsimd.tensor_tensor(cn[:], m[:], ut[:], OP.add)
                sc = epool.tile([128, NB], fp32, tag=f"sc{s}")
                nc.scalar.activation(sc[:], cn[:], AF.Sigmoid, scale=a_ap)
                sceff = epool.tile([128, NB], fp32, tag=f"sceff{s}")
                nc.vector.tensor_scalar(sceff[:], sc[:], a_ap, b_ap, OP.mult, OP.add)
                hn = hpool.tile([128, NB], fp16, tag=f"h{s}")
                nc.vector.tensor_tensor(hn[:], o_ap, sceff[:], OP.mult)

                if it == 0:
                    # L2's half of iteration 0 is garbage; its real initial
                    # state is zero.
                    nc.vector.memset(hn[64:128, :], 0.0)
                    nc.vector.memset(cn[64:128, :], 0.0)
                else:
                    nc.sync.dma_start(out_d[s, it - 1, :, :], hn[64:128, :])

                h_prev[s] = hn
                c_prev[s] = cn

    return nc


def _get_nc(has_bias: bool):
    key = ("nc", has_bias)
    if key not in _CACHE:
        _CACHE[key] = _build(has_bias)
    return _CACHE[key]


def kernel(x, W1, U1, b1, W2, U2, b2):
    global LAST_EXEC_NS
    from concourse import bass_utils

    x = np.asarray(x, dtype=np.float32)
    W1 = np.asarray(W1, dtype=np.float32)
    U1 = np.asarray(U1, dtype=np.float32)
    b1 = np.asarray(b1, dtype=np.float32)
    W2 = np.asarray(W2, dtype=np.float32)
    U2 = np.asarray(U2, dtype=np.float32)
    b2 = np.asarray(b2, dtype=np.float32)

    has_bias = bool(np.abs(b1).max() > 0 or np.abs(b2).max() > 0)

    # Host-side weight prep (gate order i,f,g,o).  Layer-1 g columns x2 so a
    # single Sigmoid computes 2*sigmoid(2*g_pre) - 1 = tanh(g_pre) after the
    # affine fixup.
    gsl = slice(2 * H, 3 * H)
    w1 = W1.copy()
    w1[:, gsl] *= 2.0
    u1 = U1.copy()
    u1[:, gsl] *= 2.0
    v2 = np.concatenate([W2, U2], axis=0)

    bias = None
    if has_bias:
        b1s = b1.copy()
        b1s[gsl] *= 2.0
        bias = np.zeros((128, 4), np.float32)
        for g in range(4):
            bias[0:64, g] = b1s[g * H : (g + 1) * H]
            bias[64:128, g] = b2[g * H : (g + 1) * H]

    nc = _get_nc(has_bias)

    wts = np.zeros((128, 3 * G4), np.float16)
    wts[:, 0:G4] = w1
    wts[0:64, G4 : 2 * G4] = u1
    wts[:, 2 * G4 : 3 * G4] = v2

    in_maps = []
    for c in range(NCORES):
        xc = x[c * BL : (c + 1) * BL]                       # [BL, T, D]
        xt = xc.reshape(NS, NB, T, D).transpose(0, 2, 3, 1)  # [NS, T, D, NB]
        m = {
            "xt": np.ascontiguousarray(xt, dtype=np.float16),
            "wts": wts,
        }
        if has_bias:
            m["bias"] = bias
        in_maps.append(m)

    _install_wait_split_patch()
    if TRACE:
        _install_ntff_shim()
    res = bass_utils.run_bass_kernel_spmd(
        nc, in_maps, core_ids=list(range(NCORES)), trace=TRACE
    )
    LAST_EXEC_NS = res.exec_time_ns

    outs = []
    for c in range(NCORES):
        o = res.results[c]["out"]            # [NS, T, H, NB] fp16
        o = o.transpose(0, 3, 1, 2)          # [NS, NB, T, H]
        outs.append(o.reshape(BL, T * H, 1))
    return np.concatenate(outs, axis=0).astype(np.float32)
